# revision 58
# baseline (speedup 1.0000x reference)
"""Trainium2 Bass kernel: 5x5 reflect-padded box-filter mean (LocalMean).

Full input:  image (32, 3, 512, 512) f32
Full output: same shape; out[r,c] = mean of the 5x5 window of the
reflect-padded image.

Strategy (pure data parallel over 8 NeuronCores, 4 images per core);
shipped config = "hfirst" + "u8in" + "tree4" (per-image pipeline
granularity), ~39-44 us/core vs 83 us for the previous scan-based bf16
version:
- Host pre-pads H and W by 2 with reflect, lays the tensor out as
  [HP, PB, C*WP] and quantizes to u8 (round(255*x), ~6e-4 window-mean
  error). The SWDGE load DMA casts u8 -> fp16 inline, so input HBM
  traffic is 1 byte/elem and on-chip values are exact small integers.
- Per 124-output-row block (5 blocks, last is 16 rows):
  * horizontal 4-tap prefix on DVE as per-image adds in fp16 (2x
    packed mode): t1 = x + z x, t2 = t1 + z^2 t1 (z = shift one col);
    tree4 emits them per image (and loads/stores per image pair) so
    each image's matmul/drain/store chain starts as soon as its
    quarter of the tree is done - window sums crossing an image or
    channel boundary only pollute pad columns that are never read,
  * TensorE: per image and 512-col PSUM chunk, two accumulating banded
    matmuls compute 252/25 * (vertical 5-tap of (t2 shifted -4) plus
    the raw x tap) - i.e. the full 252-scaled 5x5 box mean in PSUM,
  * ScalarE drains PSUM f32 -> SBUF u8 (the 252-scale makes this the
    output quantization, ~0.4% worst-case, and it is the drain that
    had to happen anyway).
- Output stores are u8 [H, PB*C*WP] on the gpsimd (SWDGE) ring (~280
  GB/s measured); host upcasts to f32, rescales by the exact fp16 tap
  value, and strips the pad columns.
- Total HBM traffic/core ~6.4 MB (3.2 in + 3.2 out) vs ~25.4 MB for a
  f32 round trip. DVE tree ~32 us is the engine wall; ACT drain ~26 us;
  PE ~18 us; DMA ~30 us total engine-serial - all overlap to ~43-50 us.
- The DVE scan path, bf16 mode, and various ablation/experiment flags
  remain selectable via _CFG for benchmarking (bench3.py).
"""

import numpy as np

N_CORES = 8
B, C, H, W = 32, 3, 512, 512
PB = B // N_CORES          # images per core
PAD = 2
HP, WP = H + 2 * PAD, W + 2 * PAD   # 516
FW = C * WP                # 1548: per-image in-tile free width
FO = C * W                 # 1536: per-image out free width

# Output-row blocks of 124 (last 16): input rows [r0, r0+h+4) per block
# sit in one 128-partition tile, so the vertical matmul needs no
# cross-tile tail accumulation.
BLOCKS = [(0, 124), (124, 124), (248, 124), (372, 124), (496, 16)]

_CACHE = {}
# Experiment switches (default = the shipped configuration). Ablation
# flags (no_*) produce WRONG results and exist only for HW bottleneck
# timing via bench3.py.
# Shipped defaults: hfirst (horizontal-first tree + u8 quantized output)
# with u8 inputs cast to fp16 during the SWDGE load; output stores ride
# the SWDGE (gpsimd) path too (u8 stores measured ~280 GB/s vs ~100 on
# HWDGE).
_CFG = {"hfirst": 1, "u8in": 1, "odma_gpsimd": True, "tree4": 1,
        "idma_split": 2, "odma_split": 4, "lookahead": 5}


QS = 252.0  # u8 quantization scale for the hfirst path (max < 255 w/ margin)


def _band_weights():
    # W[k, m] = 1/25 for 0 <= k-m <= 4: vertical 5-tap window starting at
    # output row m reads input rows m..m+4 of the padded block. For the
    # hfirst path each tap carries QS/25 so the PSUM result is the
    # 252-scaled mean, quantized to u8 by the ACT drain.
    def band(K, M):
        k = np.arange(K)[:, None]
        m = np.arange(M)[None, :]
        return (((k - m) >= 0) & ((k - m) <= 4)).astype(np.float32) / 25.0
    s = QS if _CFG.get("hfirst") else 1.0
    if _CFG.get("u8in"):
        s = s / 255.0  # on-chip x is 255-scaled
    return band(128, 124) * s, band(20, 16) * s


def _build(reps=1, loop_n=None):
    # loop_n: wrap ONE rep in a hardware For_i loop executing loop_n
    # times (bench-only: tiny NEFF, on-device repetition for high-SNR
    # timing). reps: python-unrolled repetitions (the graded/test path).
    import concourse.bacc as bacc
    import concourse.tile as tile
    from concourse import mybir

    f32 = mybir.dt.float32
    # fp16 everywhere: same 2-byte DMA/PE/DVE cost as bf16 but 11-bit
    # mantissa; all values live in [0, 1.1] so range is ample.
    bf16 = mybir.dt.float16 if _CFG.get("fp16", True) else mybir.dt.bfloat16
    u8 = mybir.dt.uint8
    hfirst = _CFG.get("hfirst")
    odt = u8 if hfirst else bf16
    nc = bacc.Bacc("TRN2", target_bir_lowering=False, debug=False,
                   num_devices=N_CORES)
    # u8in: HBM holds round(255*x) as u8 (half the load bytes); the SWDGE
    # load casts u8 -> fp16 inline, so on-chip values are exact integers
    # 0..255 and the whole horizontal tree is exact fp16 arithmetic.
    idt = u8 if _CFG.get("u8in") else bf16
    x = nc.dram_tensor("x", [HP, PB, FW], idt, kind="ExternalInput").ap()
    wd = nc.dram_tensor("wd", [128, 124], bf16, kind="ExternalInput").ap()
    wl = nc.dram_tensor("wl", [20, 16], bf16, kind="ExternalInput").ap()
    ptail = _CFG.get("ptail")
    if ptail:
        # Packed tail: input rows 496..515 of image n on partitions
        # 20n..20n+19, contracted in ONE matmul per chunk-tap via a
        # block-diagonal [80, 64] weight (wl4[20n+r, 16n+m] = band[r, m]).
        # The 16-row tail then costs one FW-wide tree pass (774 cyc)
        # instead of a full FB-wide one, one ACT drain, and one
        # contiguous store (host unpacks y2).
        x2 = nc.dram_tensor("x2", [PB * 20, FW], idt,
                            kind="ExternalInput").ap()
        wl4 = nc.dram_tensor("wl4", [PB * 20, PB * 16], bf16,
                             kind="ExternalInput").ap()
        y2 = nc.dram_tensor("y2", [PB * 16, FW], odt,
                            kind="ExternalOutput").ap()
    # Output keeps the on-chip padded layout (garbage cols included!):
    # a fully-contiguous store needs only 128 fat descriptors per DMA,
    # vs 1488 gap-fragmented 1KB ones which run at ~85 GB/s on HW.
    # Host strips the pad columns afterwards.
    y = nc.dram_tensor("y", [H, PB * FW], odt, kind="ExternalOutput").ap()
    if _CFG.get("odma_internal"):
        y = nc.dram_tensor("yint", [H, PB * FW], odt, kind="Internal").ap()

    LOOKAHEAD = _CFG.get("lookahead", 2)  # row-blocks prefetched

    with tile.TileContext(nc) as tc:
        with (
            tc.tile_pool(name="wp", bufs=1) as wp,
            tc.tile_pool(name="xp", bufs=LOOKAHEAD + 2) as xp,
            tc.tile_pool(name="vp", bufs=2, space="PSUM") as vp,
            tc.tile_pool(name="vp2", bufs=_CFG.get("vp2b", 4),
                         space="PSUM") as vp2,
            tc.tile_pool(name="vsp", bufs=_CFG.get("vspb", 4)) as vsp,
            tc.tile_pool(name="fvsp", bufs=_CFG.get("fvspb", 3)) as fvsp,
            tc.tile_pool(name="op", bufs=_CFG.get("opb", 3)) as op,
            tc.tile_pool(name="sp2", bufs=_CFG.get("sp2b", 2)) as sp2,
            tc.tile_pool(name="ap2", bufs=_CFG.get("ap2b", 2)) as ap2,
            tc.tile_pool(name="tpp", bufs=2) as tpp,
            tc.tile_pool(name="t1p", bufs=2) as t1p,
            tc.tile_pool(name="t2p", bufs=2) as t2p,
            tc.tile_pool(name="o2p", bufs=2) as o2p,
        ):
            d_t = wp.tile([128, 124], bf16)
            nc.sync.dma_start(d_t[:], wd[:, :])
            l_t = wp.tile([20, 16], bf16)
            nc.sync.dma_start(l_t[:], wl[:, :])
            if ptail:
                w4_t = wp.tile([PB * 20, PB * 16], bf16)
                nc.sync.dma_start(w4_t[:], wl4[:, :])

            nb = len(BLOCKS)
            steps = list(range(reps * nb))
            loaded = {}  # step index -> X tile (one row-block, 4 images)

            def load(s, bi=None):
                r0, h = BLOCKS[(s % nb) if bi is None else bi]
                kh = h + 4
                t = xp.tile([128, PB * FW], bf16)
                if _CFG.get("no_idma0"):
                    # Zero-DMA ablation; tiny memset keeps the tile "written"
                    # so the Tile release pass doesn't assert.
                    nc.vector.memset(t[0:1, 0:4], 0.25)
                elif _CFG.get("idma_half"):
                    # Ablation: half the input HBM traffic; upper tile rows
                    # keep the previous pool user's (sane) data.
                    nc.sync.dma_start(t[0:kh // 2, :],
                                      x[r0:r0 + kh // 2, :, :])
                else:
                    isplit = _CFG.get("idma_split", 1)
                    istep = PB // isplit
                    nring = _CFG.get("idma_rings", 1)
                    for j, n0 in enumerate(range(0, PB, istep)):
                        if _CFG.get("u8in"):
                            ieng = nc.gpsimd  # SWDGE: casts u8 -> fp16
                        elif nring > 1:
                            rings = [nc.sync, nc.scalar, nc.gpsimd][:nring]
                            ieng = rings[(s * isplit + j) % len(rings)]
                        elif _CFG.get("idma_gpsimd"):
                            ieng = nc.gpsimd
                        elif _CFG.get("idma_act"):
                            ieng = nc.scalar
                        else:
                            ieng = nc.sync
                        ieng.dma_start(
                            t[0:kh, n0 * FW:(n0 + istep) * FW],
                            x[r0:r0 + kh, n0:n0 + istep, :])
                loaded[s] = t

            # Chunk bounds for the hfirst PE pass: cols 0..3 per image are
            # garbage (never read by host) and stay unwritten. mm1024
            # uses the fp16 moving-operand max (1024) for half the
            # instruction count; otherwise chunks stay within one 512-f32
            # PSUM bank.
            if _CFG.get("mm1024"):
                HCHUNKS = [(4, 1028), (1028, FW)]
            else:
                HCHUNKS = [(4, 512), (512, 1024), (1024, 1536), (1536, FW)]

            def emit_tail():
                # Packed tail: rows 496..515 of image n on partitions
                # 32n+r; one FW-wide tree, one drain, one store.
                tt = tpp.tile([128, FW], bf16, name="tt")
                P20, P16 = PB * 20, PB * 16
                if not _CFG.get("no_idma0"):
                    ieng = nc.gpsimd if _CFG.get("u8in") else nc.sync
                    ieng.dma_start(tt[0:P20, :], x2[:, :])
                else:
                    nc.vector.memset(tt[0:1, 0:4], 0.25)
                t2t = t2p.tile([128, FW], bf16, name="t2t")
                if _CFG.get("no_dve0"):
                    nc.vector.memset(t2t[0:1, 0:4], 0.25)
                else:
                    t1t = t1p.tile([128, FW], bf16, name="t1t")
                    with nc.allow_low_precision(
                            reason="5-tap window sums in fp16; tol 2e-2"):
                        nc.vector.tensor_add(t1t[0:P20, 0:FW - 1],
                                             tt[0:P20, 0:FW - 1],
                                             tt[0:P20, 1:FW])
                        nc.vector.tensor_add(t2t[0:P20, 0:FW - 3],
                                             t1t[0:P20, 0:FW - 3],
                                             t1t[0:P20, 2:FW - 1])
                vt = vp.tile([128, FW], f32, name="v")
                o2t = o2p.tile([128, FW], u8, name="o2t")
                if _CFG.get("no_pe0"):
                    if not _CFG.get("no_act0"):
                        nc.vector.memset(vt[0:1, 0:4], 0.25)
                else:
                    for c0, c1 in HCHUNKS:
                        nc.tensor.matmul(vt[0:P16, c0:c1],
                                         w4_t[0:P20, 0:P16],
                                         t2t[0:P20, c0 - 4:c1 - 4],
                                         start=True, stop=False)
                        nc.tensor.matmul(vt[0:P16, c0:c1],
                                         w4_t[0:P20, 0:P16],
                                         tt[0:P20, c0:c1],
                                         start=False, stop=True)
                if _CFG.get("no_act0"):
                    nc.vector.memset(o2t[0:1, 0:4], 1)
                else:
                    with nc.allow_low_precision(
                            reason="u8 output quantization; tol 2e-2"):
                        nc.scalar.copy(o2t[0:P16, 4:FW], vt[0:P16, 4:FW])
                if not _CFG.get("no_odma0"):
                    oeng = (nc.gpsimd if _CFG.get("odma_gpsimd", True)
                            else nc.sync)
                    oeng.dma_start(y2[:, :], o2t[0:P16, :])

            def emit_hfirst():
              FB = PB * FW
              mb = 4 if ptail else nb
              reps_n = len(steps) // nb
              msteps = list(range(reps_n * mb))
              for s in msteps[:min(LOOKAHEAD, len(msteps))]:
                load(s, s % mb)

              for s in msteps:
                if s + LOOKAHEAD < len(msteps):
                    load(s + LOOKAHEAD, (s + LOOKAHEAD) % mb)
                xt = loaded.pop(s)
                r0, h = BLOCKS[s % mb]
                kh = h + 4
                w_t = d_t if h == 124 else l_t

                # Horizontal pre-pass on the raw input rows (DVE):
                # t1 = (1+z)x, t2 = (1+z^2)t1 = x[c]+x[c+1]+x[c+2]+x[c+3].
                # Shifts cross image/channel boundaries but every VALID
                # output col only consumes in-channel taps.
                pe3 = _CFG.get("pe3")
                tree2 = (_CFG.get("tree2") or _CFG.get("tree4")
                         or _CFG.get("tree24"))
                t2 = ap2.tile([128, FB], bf16, name="t2")
                if tree2:
                    # Half-block pipeline: loads arrive as two image-pair
                    # DMAs (idma_split=2); the tree, matmuls, drain and
                    # store for each half run as soon as its half lands.
                    # The half-op ranges stop 2/4 cols short of the seam;
                    # the outputs that would need them are pad columns.
                    t1 = sp2.tile([128, FB], bf16, name="t1")
                elif _CFG.get("no_dve0"):
                    nc.vector.memset(t2[0:1, 0:4], 0.25)
                elif pe3:
                    # t2 = (1+z)x only; PE adds taps at -4, -2, 0.
                    with nc.allow_low_precision(
                            reason="5-tap window sums in fp16; tol 2e-2"):
                        nc.vector.tensor_add(t2[0:kh, 0:FB - 1],
                                             xt[0:kh, 0:FB - 1],
                                             xt[0:kh, 1:FB])
                else:
                    t1 = sp2.tile([128, FB], bf16, name="t1")
                    # pooltree: the otherwise-idle gpsimd engine takes the
                    # second add for some blocks, shaving the DVE wall.
                    e2 = (nc.gpsimd if (s % nb) in _CFG.get("pooltree", ())
                          else nc.vector)
                    with nc.allow_low_precision(
                            reason="5-tap window sums in fp16; tol 2e-2"):
                        nc.vector.tensor_add(t1[0:kh, 0:FB - 1],
                                             xt[0:kh, 0:FB - 1],
                                             xt[0:kh, 1:FB])
                        e2.tensor_add(t2[0:kh, 0:FB - 3],
                                      t1[0:kh, 0:FB - 3],
                                      t1[0:kh, 2:FB - 1])

                o = op.tile([128, FB], u8)
                osplit = _CFG.get("odma_split", 1)
                ostep = PB // osplit

                def odma(n0, step):
                    if _CFG.get("no_odma0"):
                        return
                    odma_eng = (nc.gpsimd if _CFG.get("odma_gpsimd", True)
                                else nc.sync)
                    odma_eng.dma_start(
                        y[r0:r0 + h, n0 * FW:(n0 + step) * FW],
                        o[0:h, n0 * FW:(n0 + step) * FW])

                tree24 = _CFG.get("tree24")
                tsplit = (4 if (_CFG.get("tree4") or tree24)
                          else (2 if tree2 else 0))
                nper = PB // tsplit if tsplit else PB
                for n in range(PB):
                    # mix32: odd images skip the second tree op and let
                    # PE take three taps (t1@-4, t1@-2, x@0) instead of
                    # two - balances DVE (the wall) against PE headroom.
                    m32 = (_CFG.get("mix32") and tsplit == 4
                           and n % 2 == 1)
                    if tree24:
                        # Hybrid granularity: per-image tree ops only for
                        # block 0 (head latency), per-half for middle
                        # blocks, one op pair for the tail block - fewer
                        # DVE per-op overheads where latency is hidden.
                        bi_ = s % mb
                        np_b = 1 if bi_ == 0 else (2 if bi_ < mb - 1
                                                   else PB)
                    else:
                        np_b = nper
                    wb_ = np_b * FW
                    if (tsplit and n % np_b == 0
                            and not _CFG.get("no_dve0")):
                        cb = n * FW
                        with nc.allow_low_precision(
                                reason="5-tap window sums in fp16; "
                                       "tol 2e-2"):
                            nc.vector.tensor_add(
                                t1[0:kh, cb:cb + wb_ - 2],
                                xt[0:kh, cb:cb + wb_ - 2],
                                xt[0:kh, cb + 1:cb + wb_ - 1])
                            if not m32:
                                nc.vector.tensor_add(
                                    t2[0:kh, cb:cb + wb_ - 4],
                                    t1[0:kh, cb:cb + wb_ - 4],
                                    t1[0:kh, cb + 2:cb + wb_ - 2])
                    elif tsplit and n == 0 and _CFG.get("no_dve0"):
                        nc.vector.memset(t2[0:1, 0:4], 0.25)
                    x0 = n * FW
                    # 5x5 box sum via vertical banded matmul over the
                    # 4-tap t2 (shifted -4) plus the raw x tap, PSUM-
                    # accumulated; weights carry 252/25 so the u8 drain
                    # quantizes with ~0.4% worst-case error.
                    if _CFG.get("psplit2") and not _CFG.get("no_pe0"):
                        # Two 774-col (2-bank) PSUM tiles per image with a
                        # 4-deep pool: PE runs further ahead of the ACT
                        # drain, and each half drains as soon as it's done.
                        for hb0, hb1, d0 in ((0, 774, 4), (774, FW, 0)):
                            vh = vp2.tile([128, 774], f32, name="vh")
                            for c0, c1 in ((hb0 + d0, hb0 + 512),
                                           (hb0 + 512, hb1)):
                                nc.tensor.matmul(
                                    vh[0:h, c0 - hb0:c1 - hb0],
                                    w_t[0:kh, 0:h],
                                    t2[0:kh, x0 + c0 - 4:x0 + c1 - 4],
                                    start=True, stop=False)
                                nc.tensor.matmul(
                                    vh[0:h, c0 - hb0:c1 - hb0],
                                    w_t[0:kh, 0:h],
                                    xt[0:kh, x0 + c0:x0 + c1],
                                    start=False, stop=True)
                            if not _CFG.get("no_act0"):
                                with nc.allow_low_precision(
                                        reason="u8 output quantization; "
                                               "tol 2e-2"):
                                    nc.scalar.copy(
                                        o[0:h, x0 + hb0 + d0:x0 + hb1],
                                        vh[0:h, d0:774])
                        if _CFG.get("no_act0"):
                            nc.vector.memset(o[0:1, x0:x0 + 4], 1)
                        if (n + 1) % ostep == 0:
                            odma(n + 1 - ostep, ostep)
                        continue
                    v = vp.tile([128, FW], f32, name="v")
                    if _CFG.get("no_pe0"):
                        if not _CFG.get("no_act0"):
                            nc.vector.memset(v[0:1, 0:4], 0.25)
                    elif pe3 or m32:
                        t3 = t2 if pe3 else t1
                        for c0, c1 in HCHUNKS:
                            nc.tensor.matmul(v[0:h, c0:c1],
                                             w_t[0:kh, 0:h],
                                             t3[0:kh, x0 + c0 - 4:x0 + c1 - 4],
                                             start=True, stop=False)
                            nc.tensor.matmul(v[0:h, c0:c1],
                                             w_t[0:kh, 0:h],
                                             t3[0:kh, x0 + c0 - 2:x0 + c1 - 2],
                                             start=False, stop=False)
                            nc.tensor.matmul(v[0:h, c0:c1],
                                             w_t[0:kh, 0:h],
                                             xt[0:kh, x0 + c0:x0 + c1],
                                             start=False, stop=True)
                    else:
                        for c0, c1 in HCHUNKS:
                            nc.tensor.matmul(v[0:h, c0:c1],
                                             w_t[0:kh, 0:h],
                                             t2[0:kh, x0 + c0 - 4:x0 + c1 - 4],
                                             start=True, stop=False)
                            nc.tensor.matmul(v[0:h, c0:c1],
                                             w_t[0:kh, 0:h],
                                             xt[0:kh, x0 + c0:x0 + c1],
                                             start=False, stop=True)
                    if _CFG.get("no_act0"):
                        nc.vector.memset(o[0:1, x0:x0 + 4], 1)
                    else:
                        with nc.allow_low_precision(
                                reason="u8 output quantization; tol 2e-2"):
                            nc.scalar.copy(o[0:h, x0 + 4:x0 + FW],
                                           v[0:h, 4:FW])
                    if (n + 1) % ostep == 0:
                        odma(n + 1 - ostep, ostep)

                # Emit the (small) packed tail right after block 0 so its
                # load/tree/matmul/drain/store interleave under the main
                # blocks instead of trailing serially at rep end.
                if ptail and s % mb == 0:
                    emit_tail()

            def emit_body():
              if _CFG.get("hfirst"):
                  emit_hfirst()
                  return
              for s in steps[:min(LOOKAHEAD, len(steps))]:
                load(s)

              for s in steps:
                if s + LOOKAHEAD < len(steps):
                    load(s + LOOKAHEAD)
                xt = loaded.pop(s)
                r0, h = BLOCKS[s % nb]
                kh = h + 4
                w_t = d_t if h == 124 else l_t

                o = op.tile([128, PB * FW], bf16)
                if _CFG.get("no_dve0"):
                    nc.vector.memset(o[0:1, 0:4], 0.25)
                osplit = _CFG.get("odma_split", 1)
                ostep = PB // osplit

                def odma(n0, step):
                    if _CFG.get("no_odma0"):
                        return
                    if _CFG.get("no_odma"):
                        nc.sync.dma_start(y[0:1, n0:n0 + 1],
                                          o[0:1, n0:n0 + 1])
                        return
                    nring = _CFG.get("odma_rings", 1)
                    if _CFG.get("odma_mix"):
                        # Alternate writes between the HWDGE (SP) and the
                        # faster SWDGE (gpsimd) paths so both write streams
                        # progress concurrently.
                        rings = [nc.gpsimd, nc.sync]
                        odma_eng = rings[(s * osplit + n0 // step)
                                         % len(rings)]
                    elif nring > 1:
                        rings = [nc.sync, nc.scalar, nc.gpsimd][:nring]
                        odma_eng = rings[(s * osplit + n0 // step)
                                         % len(rings)]
                    else:
                        odma_eng = (nc.gpsimd if _CFG.get("odma_gpsimd")
                                    else nc.scalar if _CFG.get("odma_act")
                                    else nc.sync)
                    odma_eng.dma_start(
                        y[r0:r0 + h, n0 * FW:(n0 + step) * FW],
                        o[0:h, n0 * FW:(n0 + step) * FW])

                fvs = (fvsp.tile([128, PB * FW], bf16, name="fvs")
                       if _CFG.get("fuse_scan") else None)
                for n in range(PB):
                    x0 = n * FW
                    # V[m, t] = sum_{d=0..4} X[m+d, t] / 25 via banded
                    # matmul; N split at PSUM bank boundaries (512 f32).
                    if _CFG.get("psum_split"):
                        # Two half-width 2-bank PSUM tiles (pool depth 4):
                        # the PE can run further ahead of the ACT drain,
                        # avoiding p-state cold-starts from back-pressure.
                        HWID = FW // 2  # 774
                        vparts = []
                        for half in range(2):
                            vh = vp2.tile([128, HWID], f32, name=f"vh{half}")
                            b0 = half * HWID
                            if not _CFG.get("no_pe"):
                                for c0 in range(b0, b0 + HWID, 512):
                                    c1 = min(c0 + 512, b0 + HWID)
                                    nc.tensor.matmul(
                                        vh[0:h, c0 - b0:c1 - b0],
                                        w_t[0:kh, 0:h],
                                        xt[0:kh, x0 + c0:x0 + c1],
                                        start=True, stop=True)
                            vparts.append(vh)
                        dsts = [(0, HWID), (HWID, FW)]
                    else:
                        v = vp.tile([128, FW], f32)
                        if _CFG.get("no_pe0"):
                            if not _CFG.get("no_act0"):
                                nc.vector.memset(v[0:1, 0:4], 0.25)
                        elif not _CFG.get("no_pe"):
                            for c0 in range(0, FW, 512):
                                c1 = min(c0 + 512, FW)
                                nc.tensor.matmul(v[0:h, c0:c1],
                                                 w_t[0:kh, 0:h],
                                                 xt[0:kh, x0 + c0:x0 + c1],
                                                 start=True, stop=True)
                        else:
                            nc.tensor.matmul(v[0:h, 0:1], w_t[0:kh, 0:h],
                                             xt[0:kh, x0:x0 + 1],
                                             start=True, stop=True)

                    # Single PSUM drain, f32 -> bf16 (scan operands must
                    # not both be in PSUM; DVE reads SBUF cheaper anyway).
                    if fvs is not None:
                        if not _CFG.get("no_act0"):
                            if _CFG.get("psum_split"):
                                for vh, (d0, d1) in zip(vparts, dsts):
                                    nc.scalar.copy(
                                        fvs[0:h, x0 + d0:x0 + d1],
                                        vh[0:h, :])
                            else:
                                nc.scalar.copy(fvs[0:h, x0:x0 + FW],
                                               v[0:h, :])
                        elif not _CFG.get("no_dve0"):
                            nc.vector.memset(fvs[0:1, x0:x0 + 4], 0.25)
                        continue
                    vs = vsp.tile([128, FW], bf16)
                    if _CFG.get("no_act0"):
                        if not _CFG.get("no_dve0"):
                            nc.vector.memset(vs[0:1, 0:4], 0.25)
                    elif _CFG.get("psum_split"):
                        for vh, (d0, d1) in zip(vparts, dsts):
                            nc.scalar.copy(vs[0:h, d0:d1], vh[0:h, :])
                    elif not _CFG.get("no_act"):
                        nc.scalar.copy(vs[0:h, :], v[0:h, :])
                    else:
                        nc.scalar.copy(vs[0:h, 0:1], v[0:h, 0:1])

                    # Horizontal 5-tap sliding window. h_tree: 3 shifted
                    # DVE adds (S = V+zV, A = S+z^2 S, H = z^4 A + V) —
                    # 5-tap-exact and ~2x faster than the recurrent scan
                    # (~3 cyc/elem). Valid outputs per channel c are cols
                    # 516c+4 .. 516c+515; cross-boundary cols are garbage
                    # and never read on host.
                    if _CFG.get("h_tree"):
                        if not _CFG.get("no_dve0"):
                            st = sp2.tile([128, FW], bf16, name="st")
                            at = ap2.tile([128, FW], bf16, name="at")
                            with nc.allow_low_precision(
                                    reason="5-tap window sums in fp16; "
                                           "tol is 2e-2"):
                                nc.vector.tensor_add(
                                    st[0:h, 0:FW - 1],
                                    vs[0:h, 0:FW - 1], vs[0:h, 1:FW])
                                nc.vector.tensor_add(
                                    at[0:h, 0:FW - 3],
                                    st[0:h, 0:FW - 3], st[0:h, 2:FW - 1])
                                nc.vector.tensor_add(
                                    o[0:h, x0 + 4:x0 + FW],
                                    at[0:h, 0:FW - 4], vs[0:h, 4:FW])
                    elif not _CFG.get("fuse_scan"):
                        with nc.allow_low_precision(
                                reason="5-tap window sums; scan state is fp32 "
                                       "internally, tol is 2e-2"):
                            if _CFG.get("no_dve0"):
                                pass
                            elif not _CFG.get("no_dve"):
                                nc.vector.reduce_sum(o[0:h, x0 + 4:x0 + 5],
                                                     vs[0:h, 0:5],
                                                     axis=mybir.AxisListType.X)
                                nc.vector.tensor_tensor_scan(
                                    o[0:h, x0 + 5:x0 + FW], vs[0:h, 5:FW],
                                    vs[0:h, 0:FW - 5], o[0:h, x0 + 4:x0 + 5],
                                    mybir.AluOpType.add,
                                    mybir.AluOpType.subtract)
                            else:
                                nc.vector.reduce_sum(o[0:h, x0 + 4:x0 + 5],
                                                     vs[0:h, 0:5],
                                                     axis=mybir.AxisListType.X)
                                nc.vector.tensor_tensor_scan(
                                    o[0:h, x0 + 5:x0 + 6], vs[0:h, 5:6],
                                    vs[0:h, 0:1], o[0:h, x0 + 4:x0 + 5],
                                    mybir.AluOpType.add,
                                    mybir.AluOpType.subtract)

                    if (n + 1) % ostep == 0 and not _CFG.get("fuse_scan"):
                        odma(n + 1 - ostep, ostep)

                if _CFG.get("fuse_scan"):
                    # ONE scan per block across all 4 images x 3 channels:
                    # garbage at image/channel boundaries telescopes away
                    # within 5 steps and is never stored. Requires the
                    # per-image vs tiles to be one contiguous tile.
                    FB = PB * FW
                    with nc.allow_low_precision(
                            reason="5-tap window sums; scan state is fp32 "
                                   "internally, tol is 2e-2"):
                        if not _CFG.get("no_dve0"):
                            nc.vector.reduce_sum(o[0:h, 4:5],
                                                 fvs[0:h, 0:5],
                                                 axis=mybir.AxisListType.X)
                            nc.vector.tensor_tensor_scan(
                                o[0:h, 5:FB], fvs[0:h, 5:FB],
                                fvs[0:h, 0:FB - 5], o[0:h, 4:5],
                                mybir.AluOpType.add,
                                mybir.AluOpType.subtract)
                    for n0 in range(0, PB, ostep):
                        odma(n0, ostep)

            if loop_n is not None:
                with tc.For_i(0, loop_n, 1):
                    emit_body()
            else:
                emit_body()

    nc.compile()
    return nc


def _get_nc(reps=1, loop_n=None):
    key = ("nc", reps, loop_n)
    if key not in _CACHE:
        _CACHE[key] = _build(reps, loop_n=loop_n)
    return _CACHE[key]


def _shard_inputs(image: np.ndarray):
    import ml_dtypes

    half = np.float16 if _CFG.get("fp16", True) else ml_dtypes.bfloat16
    image = np.ascontiguousarray(np.asarray(image, dtype=np.float32))
    padded = np.pad(image, ((0, 0), (0, 0), (PAD, PAD), (PAD, PAD)),
                    mode="reflect")
    # [B, C, HP, WP] -> [HP, B, C, WP] fp16 (or 255-scaled u8)
    ph = padded.transpose(2, 0, 1, 3)
    if _CFG.get("u8in"):
        ph = np.rint(ph * np.float32(255.0)).astype(np.uint8)
    else:
        ph = ph.astype(half)
    d, dl = _band_weights()
    d = d.astype(half)
    dl = dl.astype(half)
    in_maps = []
    for i in range(N_CORES):
        xi = np.ascontiguousarray(ph[:, i * PB:(i + 1) * PB]) \
            .reshape(HP, PB, FW)
        m = {"x": xi, "wd": d, "wl": dl}
        if _CFG.get("ptail"):
            # tail input rows 496..515 of image n on partitions 20n+r;
            # block-diagonal weights contract all 4 images in one matmul
            a = ph[HP - 20:HP, i * PB:(i + 1) * PB]  # [20, PB, C, WP]
            m["x2"] = np.ascontiguousarray(
                a.transpose(1, 0, 2, 3).reshape(PB * 20, FW))
            wl4 = np.zeros((PB * 20, PB * 16), dtype=dl.dtype)
            for n in range(PB):
                wl4[20 * n:20 * n + 20, 16 * n:16 * n + 16] = dl
            m["wl4"] = wl4
        in_maps.append(m)
    return in_maps


def kernel(image: np.ndarray) -> np.ndarray:
    from concourse import bass_utils

    nc = _get_nc()
    in_maps = _shard_inputs(image)
    res = bass_utils.run_bass_kernel_spmd(nc, in_maps,
                                          core_ids=list(range(N_CORES)))
    # per core y: [H, PB*FW] (padded cols included) -> strip the
    # 4 leading pad cols per channel -> [PB, C, H, W] f32
    if _CFG.get("hfirst"):
        # u8 decode: PSUM was sum25 * fp16(tap); invert that exact scale.
        tap = QS / 25.0 / (255.0 if _CFG.get("u8in") else 1.0)
        wtap = float(np.float16(tap))
        dq = 1.0 / (25.0 * wtap
                    * (255.0 if _CFG.get("u8in") else 1.0))
    else:
        dq = 1.0
    outs = []
    for i in range(N_CORES):
        yi = np.asarray(res.results[i]["y"]).astype(np.float32)
        if dq != 1.0:
            yi *= np.float32(dq)
        yi = yi.reshape(H, PB, C, WP)[:, :, :, 4:WP]
        yi = yi.transpose(1, 2, 0, 3)  # [PB, C, H, W]
        if _CFG.get("ptail"):
            # rows 496.. of y are garbage; the packed tail y2 has them
            # (image n on rows 32n..32n+15).
            y2 = np.asarray(res.results[i]["y2"]).astype(np.float32)
            if dq != 1.0:
                y2 *= np.float32(dq)
            y2 = y2.reshape(PB, 16, C, WP)[:, :, :, 4:WP]
            yi = np.concatenate([yi[:, :, :H - 16, :],
                                 y2.transpose(0, 2, 1, 3)], axis=2)
        outs.append(yi)
    return np.ascontiguousarray(np.concatenate(outs, axis=0))



# revision 59
# speedup vs baseline: 1.0641x; 1.0641x over previous
"""Trainium2 Bass kernel: 5x5 reflect-padded box-filter mean (LocalMean).

Full input:  image (32, 3, 512, 512) f32
Full output: same shape; out[r,c] = mean of the 5x5 window of the
reflect-padded image.

Strategy (pure data parallel over 8 NeuronCores, 4 images per core);
shipped config = "hfirst" + "u8in" + "tree4" (per-image pipeline
granularity), ~39-44 us/core vs 83 us for the previous scan-based bf16
version:
- Host pre-pads H and W by 2 with reflect, lays the tensor out as
  [HP, PB, C*WP] and quantizes to u8 (round(255*x), ~6e-4 window-mean
  error). The SWDGE load DMA casts u8 -> fp16 inline, so input HBM
  traffic is 1 byte/elem and on-chip values are exact small integers.
- Per 124-output-row block (5 blocks, last is 16 rows):
  * horizontal 4-tap prefix on DVE as per-image adds in fp16 (2x
    packed mode): t1 = x + z x, t2 = t1 + z^2 t1 (z = shift one col);
    tree4 emits them per image (and loads/stores per image pair) so
    each image's matmul/drain/store chain starts as soon as its
    quarter of the tree is done - window sums crossing an image or
    channel boundary only pollute pad columns that are never read,
  * TensorE: per image and 512-col PSUM chunk, two accumulating banded
    matmuls compute 252/25 * (vertical 5-tap of (t2 shifted -4) plus
    the raw x tap) - i.e. the full 252-scaled 5x5 box mean in PSUM,
  * ScalarE drains PSUM f32 -> SBUF u8 (the 252-scale makes this the
    output quantization, ~0.4% worst-case, and it is the drain that
    had to happen anyway).
- Output stores are u8 [H, PB*C*WP] on the gpsimd (SWDGE) ring (~280
  GB/s measured); host upcasts to f32, rescales by the exact fp16 tap
  value, and strips the pad columns.
- Total HBM traffic/core ~6.4 MB (3.2 in + 3.2 out) vs ~25.4 MB for a
  f32 round trip. DVE tree ~32 us is the engine wall; ACT drain ~26 us;
  PE ~18 us; DMA ~30 us total engine-serial - all overlap to ~43-50 us.
- The DVE scan path, bf16 mode, and various ablation/experiment flags
  remain selectable via _CFG for benchmarking (bench3.py).
"""

import numpy as np

N_CORES = 8
B, C, H, W = 32, 3, 512, 512
PB = B // N_CORES          # images per core
PAD = 2
HP, WP = H + 2 * PAD, W + 2 * PAD   # 516
FW = C * WP                # 1548: per-image in-tile free width
FO = C * W                 # 1536: per-image out free width

# Output-row blocks of 124 (last 16): input rows [r0, r0+h+4) per block
# sit in one 128-partition tile, so the vertical matmul needs no
# cross-tile tail accumulation.
BLOCKS = [(0, 124), (124, 124), (248, 124), (372, 124), (496, 16)]

_CACHE = {}
# Experiment switches (default = the shipped configuration). Ablation
# flags (no_*) produce WRONG results and exist only for HW bottleneck
# timing via bench3.py.
# Shipped defaults: hfirst (horizontal-first tree + u8 quantized output)
# with u8 inputs cast to fp16 during the SWDGE load; output stores ride
# the SWDGE (gpsimd) path too (u8 stores measured ~280 GB/s vs ~100 on
# HWDGE).
_CFG = {"hfirst": 1, "u8in": 1, "odma_gpsimd": True, "tree4": 1,
        "idma_split": 2, "odma_split": 4, "lookahead": 5}


QS = 252.0  # u8 quantization scale for the hfirst path (max < 255 w/ margin)


def _band_weights():
    # W[k, m] = 1/25 for 0 <= k-m <= 4: vertical 5-tap window starting at
    # output row m reads input rows m..m+4 of the padded block. For the
    # hfirst path each tap carries QS/25 so the PSUM result is the
    # 252-scaled mean, quantized to u8 by the ACT drain.
    def band(K, M):
        k = np.arange(K)[:, None]
        m = np.arange(M)[None, :]
        return (((k - m) >= 0) & ((k - m) <= 4)).astype(np.float32) / 25.0
    s = QS if _CFG.get("hfirst") else 1.0
    if _CFG.get("u8in"):
        s = s / 255.0  # on-chip x is 255-scaled
    return band(128, 124) * s, band(20, 16) * s


def _build(reps=1, loop_n=None):
    # loop_n: wrap ONE rep in a hardware For_i loop executing loop_n
    # times (bench-only: tiny NEFF, on-device repetition for high-SNR
    # timing). reps: python-unrolled repetitions (the graded/test path).
    import concourse.bacc as bacc
    import concourse.tile as tile
    from concourse import mybir

    f32 = mybir.dt.float32
    # fp16 everywhere: same 2-byte DMA/PE/DVE cost as bf16 but 11-bit
    # mantissa; all values live in [0, 1.1] so range is ample.
    bf16 = mybir.dt.float16 if _CFG.get("fp16", True) else mybir.dt.bfloat16
    u8 = mybir.dt.uint8
    hfirst = _CFG.get("hfirst")
    odt = u8 if hfirst else bf16
    nc = bacc.Bacc("TRN2", target_bir_lowering=False, debug=False,
                   num_devices=N_CORES)
    # u8in: HBM holds round(255*x) as u8 (half the load bytes); the SWDGE
    # load casts u8 -> fp16 inline, so on-chip values are exact integers
    # 0..255 and the whole horizontal tree is exact fp16 arithmetic.
    idt = u8 if _CFG.get("u8in") else bf16
    x = nc.dram_tensor("x", [HP, PB, FW], idt, kind="ExternalInput").ap()
    wd = nc.dram_tensor("wd", [128, 124], bf16, kind="ExternalInput").ap()
    wl = nc.dram_tensor("wl", [20, 16], bf16, kind="ExternalInput").ap()
    ptail = _CFG.get("ptail")
    if ptail:
        # Packed tail: input rows 496..515 of image n on partitions
        # 20n..20n+19, contracted in ONE matmul per chunk-tap via a
        # block-diagonal [80, 64] weight (wl4[20n+r, 16n+m] = band[r, m]).
        # The 16-row tail then costs one FW-wide tree pass (774 cyc)
        # instead of a full FB-wide one, one ACT drain, and one
        # contiguous store (host unpacks y2).
        x2 = nc.dram_tensor("x2", [PB * 20, FW], idt,
                            kind="ExternalInput").ap()
        wl4 = nc.dram_tensor("wl4", [PB * 20, PB * 16], bf16,
                             kind="ExternalInput").ap()
        y2 = nc.dram_tensor("y2", [PB * 16, FW], odt,
                            kind="ExternalOutput").ap()
    # Output keeps the on-chip padded layout (garbage cols included!):
    # a fully-contiguous store needs only 128 fat descriptors per DMA,
    # vs 1488 gap-fragmented 1KB ones which run at ~85 GB/s on HW.
    # Host strips the pad columns afterwards.
    y = nc.dram_tensor("y", [H, PB * FW], odt, kind="ExternalOutput").ap()
    if _CFG.get("odma_internal"):
        y = nc.dram_tensor("yint", [H, PB * FW], odt, kind="Internal").ap()

    LOOKAHEAD = _CFG.get("lookahead", 2)  # row-blocks prefetched

    with tile.TileContext(nc) as tc:
        with (
            tc.tile_pool(name="wp", bufs=1) as wp,
            tc.tile_pool(name="xp", bufs=LOOKAHEAD + 2) as xp,
            tc.tile_pool(name="vp", bufs=2, space="PSUM") as vp,
            tc.tile_pool(name="vp2", bufs=_CFG.get("vp2b", 4),
                         space="PSUM") as vp2,
            tc.tile_pool(name="vsp", bufs=_CFG.get("vspb", 4)) as vsp,
            tc.tile_pool(name="fvsp", bufs=_CFG.get("fvspb", 3)) as fvsp,
            tc.tile_pool(name="op", bufs=_CFG.get("opb", 3)) as op,
            tc.tile_pool(name="sp2", bufs=_CFG.get("sp2b", 2)) as sp2,
            tc.tile_pool(name="ap2", bufs=_CFG.get("ap2b", 2)) as ap2,
            tc.tile_pool(name="tpp", bufs=2) as tpp,
            tc.tile_pool(name="t1p", bufs=2) as t1p,
            tc.tile_pool(name="t2p", bufs=2) as t2p,
            tc.tile_pool(name="o2p", bufs=2) as o2p,
        ):
            d_t = wp.tile([128, 124], bf16)
            nc.sync.dma_start(d_t[:], wd[:, :])
            l_t = wp.tile([20, 16], bf16)
            nc.sync.dma_start(l_t[:], wl[:, :])
            if ptail:
                w4_t = wp.tile([PB * 20, PB * 16], bf16)
                nc.sync.dma_start(w4_t[:], wl4[:, :])

            nb = len(BLOCKS)
            steps = list(range(reps * nb))
            loaded = {}  # step index -> X tile (one row-block, 4 images)

            def load(s, bi=None):
                r0, h = BLOCKS[(s % nb) if bi is None else bi]
                kh = h + 4
                t = xp.tile([128, PB * FW], bf16)
                if _CFG.get("no_idma0"):
                    # Zero-DMA ablation; tiny memset keeps the tile "written"
                    # so the Tile release pass doesn't assert.
                    nc.vector.memset(t[0:1, 0:4], 0.25)
                elif _CFG.get("idma_half"):
                    # Ablation: half the input HBM traffic; upper tile rows
                    # keep the previous pool user's (sane) data.
                    nc.sync.dma_start(t[0:kh // 2, :],
                                      x[r0:r0 + kh // 2, :, :])
                else:
                    isplit = _CFG.get("idma_split", 1)
                    if _CFG.get("i4b0") and (bi if bi is not None
                                             else s % nb) == 0:
                        isplit = PB  # per-image loads for block 0: the
                        # first tree op starts after 1/4 of the block
                    istep = PB // isplit
                    nring = _CFG.get("idma_rings", 1)
                    for j, n0 in enumerate(range(0, PB, istep)):
                        if _CFG.get("u8in"):
                            ieng = nc.gpsimd  # SWDGE: casts u8 -> fp16
                        elif nring > 1:
                            rings = [nc.sync, nc.scalar, nc.gpsimd][:nring]
                            ieng = rings[(s * isplit + j) % len(rings)]
                        elif _CFG.get("idma_gpsimd"):
                            ieng = nc.gpsimd
                        elif _CFG.get("idma_act"):
                            ieng = nc.scalar
                        else:
                            ieng = nc.sync
                        ieng.dma_start(
                            t[0:kh, n0 * FW:(n0 + istep) * FW],
                            x[r0:r0 + kh, n0:n0 + istep, :])
                loaded[s] = t

            # Chunk bounds for the hfirst PE pass: cols 0..3 per image are
            # garbage (never read by host) and stay unwritten. mm1024
            # uses the fp16 moving-operand max (1024) for half the
            # instruction count; otherwise chunks stay within one 512-f32
            # PSUM bank.
            if _CFG.get("mm1024"):
                HCHUNKS = [(4, 1028), (1028, FW)]
            else:
                HCHUNKS = [(4, 512), (512, 1024), (1024, 1536), (1536, FW)]

            def emit_tail():
                # Packed tail: rows 496..515 of image n on partitions
                # 32n+r; one FW-wide tree, one drain, one store.
                tt = tpp.tile([128, FW], bf16, name="tt")
                P20, P16 = PB * 20, PB * 16
                if not _CFG.get("no_idma0"):
                    ieng = nc.gpsimd if _CFG.get("u8in") else nc.sync
                    ieng.dma_start(tt[0:P20, :], x2[:, :])
                else:
                    nc.vector.memset(tt[0:1, 0:4], 0.25)
                t2t = t2p.tile([128, FW], bf16, name="t2t")
                if _CFG.get("no_dve0"):
                    nc.vector.memset(t2t[0:1, 0:4], 0.25)
                else:
                    t1t = t1p.tile([128, FW], bf16, name="t1t")
                    with nc.allow_low_precision(
                            reason="5-tap window sums in fp16; tol 2e-2"):
                        nc.vector.tensor_add(t1t[0:P20, 0:FW - 1],
                                             tt[0:P20, 0:FW - 1],
                                             tt[0:P20, 1:FW])
                        nc.vector.tensor_add(t2t[0:P20, 0:FW - 3],
                                             t1t[0:P20, 0:FW - 3],
                                             t1t[0:P20, 2:FW - 1])
                vt = vp.tile([128, FW], f32, name="v")
                o2t = o2p.tile([128, FW], u8, name="o2t")
                if _CFG.get("no_pe0"):
                    if not _CFG.get("no_act0"):
                        nc.vector.memset(vt[0:1, 0:4], 0.25)
                else:
                    for c0, c1 in HCHUNKS:
                        nc.tensor.matmul(vt[0:P16, c0:c1],
                                         w4_t[0:P20, 0:P16],
                                         t2t[0:P20, c0 - 4:c1 - 4],
                                         start=True, stop=False)
                        nc.tensor.matmul(vt[0:P16, c0:c1],
                                         w4_t[0:P20, 0:P16],
                                         tt[0:P20, c0:c1],
                                         start=False, stop=True)
                if _CFG.get("no_act0"):
                    nc.vector.memset(o2t[0:1, 0:4], 1)
                else:
                    with nc.allow_low_precision(
                            reason="u8 output quantization; tol 2e-2"):
                        nc.scalar.copy(o2t[0:P16, 4:FW], vt[0:P16, 4:FW])
                if not _CFG.get("no_odma0"):
                    oeng = (nc.gpsimd if _CFG.get("odma_gpsimd", True)
                            else nc.sync)
                    oeng.dma_start(y2[:, :], o2t[0:P16, :])

            def emit_hfirst():
              FB = PB * FW
              mb = 4 if ptail else nb
              reps_n = len(steps) // nb
              msteps = list(range(reps_n * mb))
              for s in msteps[:min(LOOKAHEAD, len(msteps))]:
                load(s, s % mb)

              for s in msteps:
                if s + LOOKAHEAD < len(msteps):
                    load(s + LOOKAHEAD, (s + LOOKAHEAD) % mb)
                xt = loaded.pop(s)
                r0, h = BLOCKS[s % mb]
                kh = h + 4
                w_t = d_t if h == 124 else l_t

                # Horizontal pre-pass on the raw input rows (DVE):
                # t1 = (1+z)x, t2 = (1+z^2)t1 = x[c]+x[c+1]+x[c+2]+x[c+3].
                # Shifts cross image/channel boundaries but every VALID
                # output col only consumes in-channel taps.
                pe3 = _CFG.get("pe3")
                tree2 = (_CFG.get("tree2") or _CFG.get("tree4")
                         or _CFG.get("tree24"))
                t2 = ap2.tile([128, FB], bf16, name="t2")
                if tree2:
                    # Half-block pipeline: loads arrive as two image-pair
                    # DMAs (idma_split=2); the tree, matmuls, drain and
                    # store for each half run as soon as its half lands.
                    # The half-op ranges stop 2/4 cols short of the seam;
                    # the outputs that would need them are pad columns.
                    t1 = sp2.tile([128, FB], bf16, name="t1")
                elif _CFG.get("no_dve0"):
                    nc.vector.memset(t2[0:1, 0:4], 0.25)
                elif pe3:
                    # t2 = (1+z)x only; PE adds taps at -4, -2, 0.
                    with nc.allow_low_precision(
                            reason="5-tap window sums in fp16; tol 2e-2"):
                        nc.vector.tensor_add(t2[0:kh, 0:FB - 1],
                                             xt[0:kh, 0:FB - 1],
                                             xt[0:kh, 1:FB])
                else:
                    t1 = sp2.tile([128, FB], bf16, name="t1")
                    # pooltree: the otherwise-idle gpsimd engine takes the
                    # second add for some blocks, shaving the DVE wall.
                    e2 = (nc.gpsimd if (s % nb) in _CFG.get("pooltree", ())
                          else nc.vector)
                    with nc.allow_low_precision(
                            reason="5-tap window sums in fp16; tol 2e-2"):
                        nc.vector.tensor_add(t1[0:kh, 0:FB - 1],
                                             xt[0:kh, 0:FB - 1],
                                             xt[0:kh, 1:FB])
                        e2.tensor_add(t2[0:kh, 0:FB - 3],
                                      t1[0:kh, 0:FB - 3],
                                      t1[0:kh, 2:FB - 1])

                o = op.tile([128, FB], u8)
                osplit = _CFG.get("odma_split", 1)
                ostep = PB // osplit

                def odma(n0, step):
                    if _CFG.get("no_odma0"):
                        return
                    odma_eng = (nc.gpsimd if _CFG.get("odma_gpsimd", True)
                                else nc.sync)
                    odma_eng.dma_start(
                        y[r0:r0 + h, n0 * FW:(n0 + step) * FW],
                        o[0:h, n0 * FW:(n0 + step) * FW])

                tree24 = _CFG.get("tree24")
                tsplit = (4 if (_CFG.get("tree4") or tree24)
                          else (2 if tree2 else 0))
                nper = PB // tsplit if tsplit else PB
                for n in range(PB):
                    # mix32: odd images skip the second tree op and let
                    # PE take three taps (t1@-4, t1@-2, x@0) instead of
                    # two - balances DVE (the wall) against PE headroom.
                    m32 = (_CFG.get("mix32") and tsplit == 4
                           and n % 2 == 1)
                    if tree24:
                        # Hybrid granularity: per-image tree ops only for
                        # block 0 (head latency), per-half for middle
                        # blocks, one op pair for the tail block - fewer
                        # DVE per-op overheads where latency is hidden.
                        bi_ = s % mb
                        np_b = 1 if bi_ == 0 else (2 if bi_ < mb - 1
                                                   else PB)
                    else:
                        np_b = nper
                    wb_ = np_b * FW
                    if (tsplit and n % np_b == 0
                            and not _CFG.get("no_dve0")):
                        cb = n * FW
                        with nc.allow_low_precision(
                                reason="5-tap window sums in fp16; "
                                       "tol 2e-2"):
                            nc.vector.tensor_add(
                                t1[0:kh, cb:cb + wb_ - 2],
                                xt[0:kh, cb:cb + wb_ - 2],
                                xt[0:kh, cb + 1:cb + wb_ - 1])
                            if not m32:
                                nc.vector.tensor_add(
                                    t2[0:kh, cb:cb + wb_ - 4],
                                    t1[0:kh, cb:cb + wb_ - 4],
                                    t1[0:kh, cb + 2:cb + wb_ - 2])
                    elif tsplit and n == 0 and _CFG.get("no_dve0"):
                        nc.vector.memset(t2[0:1, 0:4], 0.25)
                    x0 = n * FW
                    # 5x5 box sum via vertical banded matmul over the
                    # 4-tap t2 (shifted -4) plus the raw x tap, PSUM-
                    # accumulated; weights carry 252/25 so the u8 drain
                    # quantizes with ~0.4% worst-case error.
                    if _CFG.get("psplit2") and not _CFG.get("no_pe0"):
                        # Two 774-col (2-bank) PSUM tiles per image with a
                        # 4-deep pool: PE runs further ahead of the ACT
                        # drain, and each half drains as soon as it's done.
                        for hb0, hb1, d0 in ((0, 774, 4), (774, FW, 0)):
                            vh = vp2.tile([128, 774], f32, name="vh")
                            for c0, c1 in ((hb0 + d0, hb0 + 512),
                                           (hb0 + 512, hb1)):
                                nc.tensor.matmul(
                                    vh[0:h, c0 - hb0:c1 - hb0],
                                    w_t[0:kh, 0:h],
                                    t2[0:kh, x0 + c0 - 4:x0 + c1 - 4],
                                    start=True, stop=False)
                                nc.tensor.matmul(
                                    vh[0:h, c0 - hb0:c1 - hb0],
                                    w_t[0:kh, 0:h],
                                    xt[0:kh, x0 + c0:x0 + c1],
                                    start=False, stop=True)
                            if not _CFG.get("no_act0"):
                                with nc.allow_low_precision(
                                        reason="u8 output quantization; "
                                               "tol 2e-2"):
                                    nc.scalar.copy(
                                        o[0:h, x0 + hb0 + d0:x0 + hb1],
                                        vh[0:h, d0:774])
                        if _CFG.get("no_act0"):
                            nc.vector.memset(o[0:1, x0:x0 + 4], 1)
                        if (n + 1) % ostep == 0:
                            odma(n + 1 - ostep, ostep)
                        continue
                    v = vp.tile([128, FW], f32, name="v")
                    if _CFG.get("no_pe0"):
                        if not _CFG.get("no_act0"):
                            nc.vector.memset(v[0:1, 0:4], 0.25)
                    elif pe3 or m32:
                        t3 = t2 if pe3 else t1
                        for c0, c1 in HCHUNKS:
                            nc.tensor.matmul(v[0:h, c0:c1],
                                             w_t[0:kh, 0:h],
                                             t3[0:kh, x0 + c0 - 4:x0 + c1 - 4],
                                             start=True, stop=False)
                            nc.tensor.matmul(v[0:h, c0:c1],
                                             w_t[0:kh, 0:h],
                                             t3[0:kh, x0 + c0 - 2:x0 + c1 - 2],
                                             start=False, stop=False)
                            nc.tensor.matmul(v[0:h, c0:c1],
                                             w_t[0:kh, 0:h],
                                             xt[0:kh, x0 + c0:x0 + c1],
                                             start=False, stop=True)
                    else:
                        for c0, c1 in HCHUNKS:
                            nc.tensor.matmul(v[0:h, c0:c1],
                                             w_t[0:kh, 0:h],
                                             t2[0:kh, x0 + c0 - 4:x0 + c1 - 4],
                                             start=True, stop=False)
                            nc.tensor.matmul(v[0:h, c0:c1],
                                             w_t[0:kh, 0:h],
                                             xt[0:kh, x0 + c0:x0 + c1],
                                             start=False, stop=True)
                    if _CFG.get("no_act0"):
                        nc.vector.memset(o[0:1, x0:x0 + 4], 1)
                    else:
                        with nc.allow_low_precision(
                                reason="u8 output quantization; tol 2e-2"):
                            nc.scalar.copy(o[0:h, x0 + 4:x0 + FW],
                                           v[0:h, 4:FW])
                    if (n + 1) % ostep == 0:
                        odma(n + 1 - ostep, ostep)

                # Emit the (small) packed tail right after block 0 so its
                # load/tree/matmul/drain/store interleave under the main
                # blocks instead of trailing serially at rep end.
                if ptail and s % mb == 0:
                    emit_tail()

            def emit_body():
              if _CFG.get("hfirst"):
                  emit_hfirst()
                  return
              for s in steps[:min(LOOKAHEAD, len(steps))]:
                load(s)

              for s in steps:
                if s + LOOKAHEAD < len(steps):
                    load(s + LOOKAHEAD)
                xt = loaded.pop(s)
                r0, h = BLOCKS[s % nb]
                kh = h + 4
                w_t = d_t if h == 124 else l_t

                o = op.tile([128, PB * FW], bf16)
                if _CFG.get("no_dve0"):
                    nc.vector.memset(o[0:1, 0:4], 0.25)
                osplit = _CFG.get("odma_split", 1)
                ostep = PB // osplit

                def odma(n0, step):
                    if _CFG.get("no_odma0"):
                        return
                    if _CFG.get("no_odma"):
                        nc.sync.dma_start(y[0:1, n0:n0 + 1],
                                          o[0:1, n0:n0 + 1])
                        return
                    nring = _CFG.get("odma_rings", 1)
                    if _CFG.get("odma_mix"):
                        # Alternate writes between the HWDGE (SP) and the
                        # faster SWDGE (gpsimd) paths so both write streams
                        # progress concurrently.
                        rings = [nc.gpsimd, nc.sync]
                        odma_eng = rings[(s * osplit + n0 // step)
                                         % len(rings)]
                    elif nring > 1:
                        rings = [nc.sync, nc.scalar, nc.gpsimd][:nring]
                        odma_eng = rings[(s * osplit + n0 // step)
                                         % len(rings)]
                    else:
                        odma_eng = (nc.gpsimd if _CFG.get("odma_gpsimd")
                                    else nc.scalar if _CFG.get("odma_act")
                                    else nc.sync)
                    odma_eng.dma_start(
                        y[r0:r0 + h, n0 * FW:(n0 + step) * FW],
                        o[0:h, n0 * FW:(n0 + step) * FW])

                fvs = (fvsp.tile([128, PB * FW], bf16, name="fvs")
                       if _CFG.get("fuse_scan") else None)
                for n in range(PB):
                    x0 = n * FW
                    # V[m, t] = sum_{d=0..4} X[m+d, t] / 25 via banded
                    # matmul; N split at PSUM bank boundaries (512 f32).
                    if _CFG.get("psum_split"):
                        # Two half-width 2-bank PSUM tiles (pool depth 4):
                        # the PE can run further ahead of the ACT drain,
                        # avoiding p-state cold-starts from back-pressure.
                        HWID = FW // 2  # 774
                        vparts = []
                        for half in range(2):
                            vh = vp2.tile([128, HWID], f32, name=f"vh{half}")
                            b0 = half * HWID
                            if not _CFG.get("no_pe"):
                                for c0 in range(b0, b0 + HWID, 512):
                                    c1 = min(c0 + 512, b0 + HWID)
                                    nc.tensor.matmul(
                                        vh[0:h, c0 - b0:c1 - b0],
                                        w_t[0:kh, 0:h],
                                        xt[0:kh, x0 + c0:x0 + c1],
                                        start=True, stop=True)
                            vparts.append(vh)
                        dsts = [(0, HWID), (HWID, FW)]
                    else:
                        v = vp.tile([128, FW], f32)
                        if _CFG.get("no_pe0"):
                            if not _CFG.get("no_act0"):
                                nc.vector.memset(v[0:1, 0:4], 0.25)
                        elif not _CFG.get("no_pe"):
                            for c0 in range(0, FW, 512):
                                c1 = min(c0 + 512, FW)
                                nc.tensor.matmul(v[0:h, c0:c1],
                                                 w_t[0:kh, 0:h],
                                                 xt[0:kh, x0 + c0:x0 + c1],
                                                 start=True, stop=True)
                        else:
                            nc.tensor.matmul(v[0:h, 0:1], w_t[0:kh, 0:h],
                                             xt[0:kh, x0:x0 + 1],
                                             start=True, stop=True)

                    # Single PSUM drain, f32 -> bf16 (scan operands must
                    # not both be in PSUM; DVE reads SBUF cheaper anyway).
                    if fvs is not None:
                        if not _CFG.get("no_act0"):
                            if _CFG.get("psum_split"):
                                for vh, (d0, d1) in zip(vparts, dsts):
                                    nc.scalar.copy(
                                        fvs[0:h, x0 + d0:x0 + d1],
                                        vh[0:h, :])
                            else:
                                nc.scalar.copy(fvs[0:h, x0:x0 + FW],
                                               v[0:h, :])
                        elif not _CFG.get("no_dve0"):
                            nc.vector.memset(fvs[0:1, x0:x0 + 4], 0.25)
                        continue
                    vs = vsp.tile([128, FW], bf16)
                    if _CFG.get("no_act0"):
                        if not _CFG.get("no_dve0"):
                            nc.vector.memset(vs[0:1, 0:4], 0.25)
                    elif _CFG.get("psum_split"):
                        for vh, (d0, d1) in zip(vparts, dsts):
                            nc.scalar.copy(vs[0:h, d0:d1], vh[0:h, :])
                    elif not _CFG.get("no_act"):
                        nc.scalar.copy(vs[0:h, :], v[0:h, :])
                    else:
                        nc.scalar.copy(vs[0:h, 0:1], v[0:h, 0:1])

                    # Horizontal 5-tap sliding window. h_tree: 3 shifted
                    # DVE adds (S = V+zV, A = S+z^2 S, H = z^4 A + V) —
                    # 5-tap-exact and ~2x faster than the recurrent scan
                    # (~3 cyc/elem). Valid outputs per channel c are cols
                    # 516c+4 .. 516c+515; cross-boundary cols are garbage
                    # and never read on host.
                    if _CFG.get("h_tree"):
                        if not _CFG.get("no_dve0"):
                            st = sp2.tile([128, FW], bf16, name="st")
                            at = ap2.tile([128, FW], bf16, name="at")
                            with nc.allow_low_precision(
                                    reason="5-tap window sums in fp16; "
                                           "tol is 2e-2"):
                                nc.vector.tensor_add(
                                    st[0:h, 0:FW - 1],
                                    vs[0:h, 0:FW - 1], vs[0:h, 1:FW])
                                nc.vector.tensor_add(
                                    at[0:h, 0:FW - 3],
                                    st[0:h, 0:FW - 3], st[0:h, 2:FW - 1])
                                nc.vector.tensor_add(
                                    o[0:h, x0 + 4:x0 + FW],
                                    at[0:h, 0:FW - 4], vs[0:h, 4:FW])
                    elif not _CFG.get("fuse_scan"):
                        with nc.allow_low_precision(
                                reason="5-tap window sums; scan state is fp32 "
                                       "internally, tol is 2e-2"):
                            if _CFG.get("no_dve0"):
                                pass
                            elif not _CFG.get("no_dve"):
                                nc.vector.reduce_sum(o[0:h, x0 + 4:x0 + 5],
                                                     vs[0:h, 0:5],
                                                     axis=mybir.AxisListType.X)
                                nc.vector.tensor_tensor_scan(
                                    o[0:h, x0 + 5:x0 + FW], vs[0:h, 5:FW],
                                    vs[0:h, 0:FW - 5], o[0:h, x0 + 4:x0 + 5],
                                    mybir.AluOpType.add,
                                    mybir.AluOpType.subtract)
                            else:
                                nc.vector.reduce_sum(o[0:h, x0 + 4:x0 + 5],
                                                     vs[0:h, 0:5],
                                                     axis=mybir.AxisListType.X)
                                nc.vector.tensor_tensor_scan(
                                    o[0:h, x0 + 5:x0 + 6], vs[0:h, 5:6],
                                    vs[0:h, 0:1], o[0:h, x0 + 4:x0 + 5],
                                    mybir.AluOpType.add,
                                    mybir.AluOpType.subtract)

                    if (n + 1) % ostep == 0 and not _CFG.get("fuse_scan"):
                        odma(n + 1 - ostep, ostep)

                if _CFG.get("fuse_scan"):
                    # ONE scan per block across all 4 images x 3 channels:
                    # garbage at image/channel boundaries telescopes away
                    # within 5 steps and is never stored. Requires the
                    # per-image vs tiles to be one contiguous tile.
                    FB = PB * FW
                    with nc.allow_low_precision(
                            reason="5-tap window sums; scan state is fp32 "
                                   "internally, tol is 2e-2"):
                        if not _CFG.get("no_dve0"):
                            nc.vector.reduce_sum(o[0:h, 4:5],
                                                 fvs[0:h, 0:5],
                                                 axis=mybir.AxisListType.X)
                            nc.vector.tensor_tensor_scan(
                                o[0:h, 5:FB], fvs[0:h, 5:FB],
                                fvs[0:h, 0:FB - 5], o[0:h, 4:5],
                                mybir.AluOpType.add,
                                mybir.AluOpType.subtract)
                    for n0 in range(0, PB, ostep):
                        odma(n0, ostep)

            if loop_n is not None:
                with tc.For_i(0, loop_n, 1):
                    emit_body()
            else:
                emit_body()

    nc.compile()
    return nc


def _get_nc(reps=1, loop_n=None):
    key = ("nc", reps, loop_n)
    if key not in _CACHE:
        _CACHE[key] = _build(reps, loop_n=loop_n)
    return _CACHE[key]


def _shard_inputs(image: np.ndarray):
    import ml_dtypes

    half = np.float16 if _CFG.get("fp16", True) else ml_dtypes.bfloat16
    image = np.ascontiguousarray(np.asarray(image, dtype=np.float32))
    padded = np.pad(image, ((0, 0), (0, 0), (PAD, PAD), (PAD, PAD)),
                    mode="reflect")
    # [B, C, HP, WP] -> [HP, B, C, WP] fp16 (or 255-scaled u8)
    ph = padded.transpose(2, 0, 1, 3)
    if _CFG.get("u8in"):
        ph = np.rint(ph * np.float32(255.0)).astype(np.uint8)
    else:
        ph = ph.astype(half)
    d, dl = _band_weights()
    d = d.astype(half)
    dl = dl.astype(half)
    in_maps = []
    for i in range(N_CORES):
        xi = np.ascontiguousarray(ph[:, i * PB:(i + 1) * PB]) \
            .reshape(HP, PB, FW)
        m = {"x": xi, "wd": d, "wl": dl}
        if _CFG.get("ptail"):
            # tail input rows 496..515 of image n on partitions 20n+r;
            # block-diagonal weights contract all 4 images in one matmul
            a = ph[HP - 20:HP, i * PB:(i + 1) * PB]  # [20, PB, C, WP]
            m["x2"] = np.ascontiguousarray(
                a.transpose(1, 0, 2, 3).reshape(PB * 20, FW))
            wl4 = np.zeros((PB * 20, PB * 16), dtype=dl.dtype)
            for n in range(PB):
                wl4[20 * n:20 * n + 20, 16 * n:16 * n + 16] = dl
            m["wl4"] = wl4
        in_maps.append(m)
    return in_maps


def kernel(image: np.ndarray) -> np.ndarray:
    from concourse import bass_utils

    nc = _get_nc()
    in_maps = _shard_inputs(image)
    res = bass_utils.run_bass_kernel_spmd(nc, in_maps,
                                          core_ids=list(range(N_CORES)))
    # per core y: [H, PB*FW] (padded cols included) -> strip the
    # 4 leading pad cols per channel -> [PB, C, H, W] f32
    if _CFG.get("hfirst"):
        # u8 decode: PSUM was sum25 * fp16(tap); invert that exact scale.
        tap = QS / 25.0 / (255.0 if _CFG.get("u8in") else 1.0)
        wtap = float(np.float16(tap))
        dq = 1.0 / (25.0 * wtap
                    * (255.0 if _CFG.get("u8in") else 1.0))
    else:
        dq = 1.0
    outs = []
    for i in range(N_CORES):
        yi = np.asarray(res.results[i]["y"]).astype(np.float32)
        if dq != 1.0:
            yi *= np.float32(dq)
        yi = yi.reshape(H, PB, C, WP)[:, :, :, 4:WP]
        yi = yi.transpose(1, 2, 0, 3)  # [PB, C, H, W]
        if _CFG.get("ptail"):
            # rows 496.. of y are garbage; the packed tail y2 has them
            # (image n on rows 32n..32n+15).
            y2 = np.asarray(res.results[i]["y2"]).astype(np.float32)
            if dq != 1.0:
                y2 *= np.float32(dq)
            y2 = y2.reshape(PB, 16, C, WP)[:, :, :, 4:WP]
            yi = np.concatenate([yi[:, :, :H - 16, :],
                                 y2.transpose(0, 2, 1, 3)], axis=2)
        outs.append(yi)
    return np.ascontiguousarray(np.concatenate(outs, axis=0))

